# revision 14
# baseline (speedup 1.0000x reference)
"""Trainium2 kernel for nn_Attention (B=8, S=2048, D=768, H=12, DH=64, R=64).

Sharding: data-parallel over batch -> 1 batch element per NeuronCore (8 cores).
No collectives. LayerNorm affine folded into QKV weights on the host; the v
bias is folded into the adapter biases (exact); q/k biases applied on device.

Per-core dataflow (all matmuls bf16, fp32 PSUM):
  y [2048,768] --LN--> x --PE transpose--> xT [d,s]
  xT -> kT [j,s], v [s,j] blocks, qT chunk 0; q projections for chunks 1-3
  are deferred into the attention phase (which is exp/ACT-bound) as inserts.
  attention per head-pair hp, s-chunk cs (512):
    per t-tile: scoresT [t,1024] = both heads via quadrant-packed matmuls
    -> one exp on ACT -> ctx accumulation (lagged 2 t-tiles for pipelining)
    v blocks are [vA|onesA|vB|onesB] (130 wide): the ones columns give each
    head's softmax denominator at psum row 64 for free.
    normalize: reciprocal_approx_fast of denom row, gpsimd partition
    broadcast, DVE mul; head B staged in SBUF then SBUF->SBUF DMA into
    ctxT partitions 64:127 (engines cannot partition-shift element ops).
  W1 = a1_w @ msa_w fused on host: hT = relu(W1 ctxT + b1) independent of msa.
  msa natural [s,e] via lhsT=ctxT; adapter accumulated into the same psum
  tile (hT_aug ones-row trick adds a2 bias); copied to SBUF and DMA'd out.
  Phase-C work for chunk c-1 is interleaved into chunk c's attention.
"""

import sys

sys.path.insert(0, "/opt/trn_rl_repo")

import numpy as np

import concourse.bass as bass
import concourse.mybir as mybir
import concourse.tile as tile
from concourse import bacc, bass_utils
from concourse.masks import make_identity

F32 = mybir.dt.float32
BF16 = mybir.dt.bfloat16

B, S, D = 8, 2048, 768
H, DH, R = 12, 64, 64
EPS = 1e-6
NCORES = 8

ST = S // 128          # 16 s-tiles
DT = D // 128          # 6 d-tiles
NP = H // 2            # 6 head pairs
PW = 130               # per-pair v block: [vA(64)|onesA | vB(64)|onesB]
EXP = mybir.ActivationFunctionType.Exp
RELU = mybir.ActivationFunctionType.Relu
SQRT = mybir.ActivationFunctionType.Sqrt


def build_nc() -> bass.Bass:
    nc = bacc.Bacc(None, target_bir_lowering=False, debug=False)

    y_ext = nc.declare_dram_parameter("y", [S, D], F32, isOutput=False)
    qkvT_ext = nc.declare_dram_parameter("qkv_wT", [D, 3 * D], BF16, isOutput=False)
    qkb_ext = nc.declare_dram_parameter("qk_b", [2 * D], F32, isOutput=False)
    msaT_ext = nc.declare_dram_parameter("msa_wT", [D, D], BF16, isOutput=False)
    w1T_ext = nc.declare_dram_parameter("w1T", [D, R], BF16, isOutput=False)
    a1b_ext = nc.declare_dram_parameter("a1_b_eff", [R], F32, isOutput=False)
    a2T_ext = nc.declare_dram_parameter("a2_wT_aug", [128, D], BF16, isOutput=False)
    out_ext = nc.declare_dram_parameter("out", [S, D], F32, isOutput=True)

    with tile.TileContext(nc) as tc:
        _build(tc, y_ext, qkvT_ext, qkb_ext, msaT_ext, w1T_ext, a1b_ext,
               a2T_ext, out_ext)
    nc.compile()
    return nc


def _build(tc, y_ext, qkvT_ext, qkb_ext, msaT_ext, w1T_ext, a1b_ext,
           a2T_ext, out_ext):
    from contextlib import ExitStack

    nc = tc.nc

    with ExitStack() as stack:
        ec = stack.enter_context
        consts = ec(tc.tile_pool(name="consts", bufs=1))
        big = ec(tc.tile_pool(name="big", bufs=1))
        temps = ec(tc.tile_pool(name="temps", bufs=2))
        small = ec(tc.tile_pool(name="small", bufs=4))

        # ---------------- constants / weights ----------------
        ident = consts.tile([128, 128], BF16)
        make_identity(nc, ident)

        msaT_sb = consts.tile([128, DT, D], BF16)
        nc.sync.dma_start(
            out=msaT_sb, in_=msaT_ext.ap().rearrange("(ko p) j -> p ko j", p=128)
        )
        w1T_sb = consts.tile([128, DT, R], BF16)
        nc.sync.dma_start(
            out=w1T_sb, in_=w1T_ext.ap().rearrange("(ko p) j -> p ko j", p=128)
        )
        a2T_sb = consts.tile([128, D], BF16)
        nc.sync.dma_start(out=a2T_sb, in_=a2T_ext.ap())

        qkb_sb = consts.tile([128, 2 * NP], F32)
        nc.sync.dma_start(
            out=qkb_sb, in_=qkb_ext.ap().rearrange("(jt p) -> p jt", p=128)
        )
        a1b_sb = consts.tile([64, 1], F32)
        nc.sync.dma_start(out=a1b_sb, in_=a1b_ext.ap()[:, None])
        eps_sb = consts.tile([128, 1], F32)
        nc.vector.memset(eps_sb, EPS)

        # ---------------- long-lived activations ----------------
        qT_sb = big.tile([128, NP, S], BF16, tag="qT")
        kT_sb = big.tile([128, NP, S], BF16, tag="kT")
        v_sb = big.tile([128, ST, NP * PW], BF16, tag="v")
        ctxT_sb = big.tile([128, DT, S], BF16, tag="ctxT")
        xT_sb = big.tile([128, DT, S], BF16, tag="xT")
        qw_sb = big.tile([128, DT, D], BF16, tag="qw")
        nc.sync.dma_start(
            out=qw_sb,
            in_=qkvT_ext.ap()[:, 0:D].rearrange("(ko p) j -> p ko j", p=128),
        )

        # ones columns at offsets 64 and 129 of each 130-wide pair block
        v_pairs = v_sb.rearrange("p t (h u) -> p t h u", u=PW)
        nc.vector.memset(v_pairs[:, :, :, 64:65], 1.0)
        nc.vector.memset(v_pairs[:, :, :, 129:130], 1.0)

        def q_proj(sc, jts, psum_pool, tag):
            for jt in jts:
                qp = psum_pool.tile([128, 512], F32, tag=tag)
                for kd in range(DT):
                    nc.tensor.matmul(
                        qp,
                        lhsT=qw_sb[:, kd, jt * 128:(jt + 1) * 128],
                        rhs=xT_sb[:, kd, sc * 512:(sc + 1) * 512],
                        start=(kd == 0), stop=(kd == DT - 1),
                    )
                nc.vector.tensor_scalar_add(
                    out=qT_sb[:, jt, sc * 512:(sc + 1) * 512],
                    in0=qp, scalar1=qkb_sb[:, jt:jt + 1],
                )

        # ------- phase 1+2: LayerNorm + transpose + k/v (+q chunk 0) -------
        kvw_sb = big.tile([128, DT, 2 * D], BF16, tag="kvw")
        nc.sync.dma_start(
            out=kvw_sb,
            in_=qkvT_ext.ap()[:, D:3 * D].rearrange("(ko p) j -> p ko j", p=128),
        )

        with tc.tile_pool(name="ph12_psum", bufs=4, space="PSUM") as psum12, \
             tc.tile_pool(name="tr_psum", bufs=3, space="PSUM") as psum_tr:

            def ln_transpose(st):
                y_t = temps.tile([128, D], F32, tag="y")
                nc.sync.dma_start(out=y_t, in_=y_ext[st * 128:(st + 1) * 128, :])
                stats = small.tile([128, 3, 6], F32, tag="stats")
                y_grp = y_t.rearrange("p (g c) -> p g c", g=3)
                for g in range(3):
                    nc.vector.bn_stats(out=stats[:, g, :], in_=y_grp[:, g, :])
                mv = small.tile([128, 2], F32, tag="mv")
                nc.vector.bn_aggr(out=mv, in_=stats)
                rstd = small.tile([128, 1], F32, tag="rstd")
                nc.scalar.activation(out=rstd, in_=mv[:, 1:2], func=SQRT,
                                     bias=eps_sb, scale=1.0)
                nc.vector.reciprocal(out=rstd, in_=rstd)
                x_bf = temps.tile([128, D], BF16, tag="xbf")
                nc.vector.tensor_scalar(
                    out=x_bf, in0=y_t, scalar1=mv[:, 0:1], scalar2=rstd,
                    op0=mybir.AluOpType.subtract, op1=mybir.AluOpType.mult,
                )
                for dt in range(DT):
                    tr = psum_tr.tile([128, 128], BF16, tag="tr")
                    nc.tensor.transpose(tr, x_bf[:, dt * 128:(dt + 1) * 128], ident)
                    nc.scalar.copy(
                        out=xT_sb[:, dt, st * 128:(st + 1) * 128], in_=tr)

            def v_proj(st):
                # v psum cols = 8 heads x 64 (jc0) then 4 heads x 64 (jc1)
                v_st = v_sb[:, st, :]
                for j0, jw, h0 in ((0, 512, 0), (512, 256, 8)):
                    vp = psum12.tile([128, 512], F32, tag="mm")
                    for kd in range(DT):
                        nc.tensor.matmul(
                            vp[:, :jw],
                            lhsT=xT_sb[:, kd, st * 128:(st + 1) * 128],
                            rhs=kvw_sb[:, kd, D + j0: D + j0 + jw],
                            start=(kd == 0), stop=(kd == DT - 1),
                        )
                    nh = jw // 64
                    dst = bass.AP(
                        tensor=v_st.tensor, offset=v_st.offset + (h0 // 2) * PW,
                        ap=[list(v_st.ap[0]), [PW, nh // 2], [65, 2], [1, 64]],
                    )
                    nc.vector.tensor_copy(
                        out=dst,
                        in_=vp[:, :jw].rearrange("p (a b e) -> p a b e",
                                                 a=nh // 2, b=2),
                    )

            def k_proj(jk, scs, psum_pool, tag):
                for sc in scs:
                    qp = psum_pool.tile([128, 512], F32, tag=tag)
                    for kd in range(DT):
                        nc.tensor.matmul(
                            qp,
                            lhsT=kvw_sb[:, kd, jk * 128:(jk + 1) * 128],
                            rhs=xT_sb[:, kd, sc * 512:(sc + 1) * 512],
                            start=(kd == 0), stop=(kd == DT - 1),
                        )
                    nc.vector.tensor_scalar_add(
                        out=kT_sb[:, jk, sc * 512:(sc + 1) * 512],
                        in0=qp, scalar1=qkb_sb[:, NP + jk:NP + jk + 1],
                    )

            # Minimal prefix before attention: all LN/transpose + v, then
            # k for head-pair 0 (all chunks) and q head-pair 0 chunk 0.
            # Everything else is deferred into attention-phase inserts
            # (the attention steady state is exp/ACT-bound).
            for st in range(ST):
                ln_transpose(st)
                if st >= 2:
                    v_proj(st - 2)
            v_proj(ST - 2)
            v_proj(ST - 1)
            k_proj(0, range(4), psum12, "mm")
            q_proj(0, (0,), psum12, "mm")

        # ------- phase 3+4: attention fused with msa/adapter per chunk -----
        with tc.tile_pool(name="et", bufs=3) as et_pool, \
             tc.tile_pool(name="rec", bufs=2) as rec_pool, \
             tc.tile_pool(name="ph4", bufs=1) as ph4, \
             tc.tile_pool(name="outp", bufs=2) as outp, \
             tc.tile_pool(name="sc_psum", bufs=2, space="PSUM") as psum_sc, \
             tc.tile_pool(name="cx_psum", bufs=2, space="PSUM") as psum_cx:

            hT_sb = ph4.tile([128, S], BF16, tag="hT")
            nc.vector.memset(hT_sb[64:128, :], 0.0)
            nc.vector.memset(hT_sb[64:65, :], 1.0)

            def phase_c_slices(c):
                """Phase C for chunk c as a list of emission closures."""
                cs = c * 512

                def s_hT():
                    hp_ = psum_sc.tile([128, 512], F32, tag="sc")
                    for kd in range(DT):
                        nc.tensor.matmul(
                            hp_[0:64, :],
                            lhsT=w1T_sb[:, kd, :],
                            rhs=ctxT_sb[:, kd, cs:cs + 512],
                            start=(kd == 0), stop=(kd == DT - 1),
                        )
                    nc.scalar.activation(
                        out=hT_sb[0:64, cs:cs + 512], in_=hp_[0:64, :],
                        func=RELU, bias=a1b_sb, scale=1.0,
                    )

                def s_out(st):
                    def emit():
                        mo = psum_sc.tile([128, D], F32, tag="sc")
                        for j0, jw in ((0, 512), (512, 256)):
                            for kd in range(DT):
                                nc.tensor.matmul(
                                    mo[:, j0:j0 + jw],
                                    lhsT=ctxT_sb[:, kd, st * 128:(st + 1) * 128],
                                    rhs=msaT_sb[:, kd, j0:j0 + jw],
                                    start=(kd == 0), stop=False,
                                )
                            nc.tensor.matmul(
                                mo[:, j0:j0 + jw],
                                lhsT=hT_sb[:, st * 128:(st + 1) * 128],
                                rhs=a2T_sb[:, j0:j0 + jw],
                                start=False, stop=True,
                            )
                        o_t = outp.tile([128, D], F32, tag="o")
                        nc.vector.tensor_copy(out=o_t, in_=mo)
                        nc.sync.dma_start(
                            out=out_ext[st * 128:(st + 1) * 128, :], in_=o_t)
                    return emit

                return [s_hT] + [s_out(st) for st in range(4 * c, 4 * c + 4)]

            def attention_hp(c, hp, inserts):
                pos = ((), (8,), (5, 11), (3, 7, 11), (3, 6, 9, 12))[len(inserts)]
                ins_at = dict(zip(pos, inserts))
                cs = c * 512
                base = hp * PW
                pA = psum_cx.tile([128, 512], F32, tag="cxA")
                pB = psum_cx.tile([128, 512], F32, tag="cxB")
                e_tiles = [None] * ST

                def ctx_pair(t):
                    nc.tensor.matmul(
                        pA[0:65, :],
                        lhsT=v_sb[:, t, base:base + 65],
                        rhs=e_tiles[t][:, 0:512],
                        start=(t == 0), stop=(t == ST - 1),
                    )
                    nc.tensor.matmul(
                        pB[0:65, :],
                        lhsT=v_sb[:, t, base + 65:base + PW],
                        rhs=e_tiles[t][:, 512:1024],
                        start=(t == 0), stop=(t == ST - 1),
                    )

                for t in range(ST):
                    sc_t = psum_sc.tile([128, 1024], F32, tag="sc")
                    nc.tensor.matmul(
                        sc_t[:, 0:512],
                        lhsT=kT_sb[0:64, hp, t * 128:(t + 1) * 128],
                        rhs=qT_sb[0:64, hp, cs:cs + 512],
                        start=True, stop=True, tile_position=(0, 0),
                    )
                    nc.tensor.matmul(
                        sc_t[:, 512:1024],
                        lhsT=kT_sb[64:128, hp, t * 128:(t + 1) * 128],
                        rhs=qT_sb[64:128, hp, cs:cs + 512],
                        start=True, stop=True, tile_position=(64, 0),
                    )
                    e_t = et_pool.tile([128, 1024], BF16, tag="e")
                    nc.scalar.activation(
                        out=e_t, in_=sc_t, func=EXP,
                        scale=float(1.0 / np.sqrt(DH)),
                    )
                    e_tiles[t] = e_t
                    if t >= 2:
                        ctx_pair(t - 2)
                    if t in ins_at:
                        ins_at[t]()
                ctx_pair(ST - 2)
                ctx_pair(ST - 1)

                # normalize: pX = [vX|ones] -> ctx rows 0:64, denom row 64.
                rrA = rec_pool.tile([1, 512], F32, tag="rrA")
                nc.vector.reciprocal(out=rrA, in_=pA[64:65, :])
                rrB = rec_pool.tile([1, 512], F32, tag="rrB")
                nc.vector.reciprocal(out=rrB, in_=pB[64:65, :])
                rA = rec_pool.tile([64, 512], F32, tag="rA")
                nc.gpsimd.partition_broadcast(rA, rrA)
                rB = rec_pool.tile([64, 512], F32, tag="rB")
                nc.gpsimd.partition_broadcast(rB, rrB)
                nc.vector.tensor_mul(
                    out=ctxT_sb[0:64, hp, cs:cs + 512],
                    in0=pA[0:64, :], in1=rA,
                )
                stgB = rec_pool.tile([64, 512], BF16, tag="stgB")
                nc.vector.tensor_mul(out=stgB, in0=pB[0:64, :], in1=rB)
                nc.gpsimd.dma_start(
                    out=ctxT_sb[64:128, hp, cs:cs + 512], in_=stgB)

            def K2(jk, scs):
                return lambda: k_proj(jk, scs, psum_sc, "sc")

            def Q1(jt, sc):
                return lambda: q_proj(sc, (jt,), psum_sc, "sc")

            # Deferred-work schedule: each (c, hp) gets up to 4 closures,
            # emitted at fixed t positions inside that hp's t-loop. Deadlines:
            # k(jk h) + q(jt h, sc0) before (0, h); q(jt j, sc) before
            # (c_sc, j); phase-C of chunk c spread over chunk c+1.
            pc = [None] + [phase_c_slices(c) for c in range(3)]
            sched = {
                (0, 0): [K2(1, (0, 1)), K2(1, (2, 3)), Q1(1, 0)],
                (0, 1): [K2(2, (0, 1)), K2(2, (2, 3)), Q1(2, 0)],
                (0, 2): [K2(3, (0, 1)), K2(3, (2, 3)), Q1(3, 0)],
                (0, 3): [K2(4, (0, 1)), K2(4, (2, 3)), Q1(4, 0)],
                (0, 4): [K2(5, (0, 1)), K2(5, (2, 3)), Q1(5, 0)],
                (0, 5): [Q1(0, 1), Q1(1, 1), Q1(2, 1)],
                (1, 0): [Q1(3, 1), pc[1][0]],
                (1, 1): [pc[1][1], Q1(4, 1)],
                (1, 2): [pc[1][2], Q1(5, 1)],
                (1, 3): [pc[1][3], Q1(0, 2)],
                (1, 4): [pc[1][4], Q1(1, 2)],
                (1, 5): [Q1(2, 2), Q1(3, 2)],
                (2, 0): [Q1(4, 2), pc[2][0]],
                (2, 1): [pc[2][1], Q1(5, 2)],
                (2, 2): [pc[2][2], Q1(0, 3)],
                (2, 3): [pc[2][3], Q1(1, 3)],
                (2, 4): [pc[2][4], Q1(2, 3)],
                (2, 5): [Q1(3, 3), Q1(4, 3), Q1(5, 3)],
                (3, 0): [pc[3][0]],
                (3, 1): [pc[3][1]],
                (3, 2): [pc[3][2]],
                (3, 3): [pc[3][3]],
                (3, 4): [pc[3][4]],
                (3, 5): [],
            }
            for c in range(4):
                for hp in range(NP):
                    attention_hp(c, hp, sched[(c, hp)])
            for emit in phase_c_slices(3):
                emit()


_NC_CACHE = None


def _get_nc():
    global _NC_CACHE
    if _NC_CACHE is None:
        _NC_CACHE = build_nc()
    return _NC_CACHE


def _prep_in_maps(y, ln_g, ln_b, qkv_w, qkv_b, msa_w, a1_w, a1_b, a2_w, a2_b):
    f = np.float32
    y = np.asarray(y, f)
    ln_g = np.asarray(ln_g, f)
    ln_b = np.asarray(ln_b, f)
    qkv_w = np.asarray(qkv_w, f)
    qkv_b = np.asarray(qkv_b, f)
    msa_w = np.asarray(msa_w, f)
    a1_w = np.asarray(a1_w, f)
    a1_b = np.asarray(a1_b, f)
    a2_w = np.asarray(a2_w, f)
    a2_b = np.asarray(a2_b, f)

    import ml_dtypes
    bf = ml_dtypes.bfloat16

    # Fold LN affine into QKV: (g*xn + b) @ W.T + c == xn @ (W*g).T + (W@b + c)
    qkv_wT = np.ascontiguousarray((qkv_w * ln_g[None, :]).T).astype(bf)
    qkv_b_eff = (qkv_b + qkv_w @ ln_b).astype(f)          # [2304]
    qk_b = np.ascontiguousarray(qkv_b_eff[: 2 * D])       # [1536]
    bv = qkv_b_eff[2 * D:]                                # [768] v bias, folded

    msa_wT = np.ascontiguousarray(msa_w.T).astype(bf)     # [768, 768]
    w1 = a1_w @ msa_w                                     # [64, 768]
    w1T = np.ascontiguousarray(w1.T).astype(bf)           # [768, 64]
    a1_b_eff = (a1_b + w1 @ bv).astype(f)                 # [64]
    a2_b_eff = (a2_b + msa_w @ bv).astype(f)              # [768]
    a2_aug = np.zeros((128, D), f)
    a2_aug[:R] = a2_w.T
    a2_aug[R] = a2_b_eff
    a2_aug = a2_aug.astype(bf)

    shared = {
        "qkv_wT": qkv_wT, "qk_b": qk_b, "msa_wT": msa_wT,
        "w1T": w1T, "a1_b_eff": a1_b_eff, "a2_wT_aug": a2_aug,
    }
    in_maps = [dict(shared, y=np.ascontiguousarray(y[b])) for b in range(NCORES)]
    return in_maps


def run(trace=False, **inputs):
    in_maps = _prep_in_maps(**inputs)
    nc = _get_nc()
    res = bass_utils.run_bass_kernel_spmd(
        nc, in_maps, core_ids=list(range(NCORES)), trace=trace
    )
    out = np.stack([r["out"] for r in res.results], axis=0)
    return out.astype(np.float32), res


def kernel(**inputs) -> np.ndarray:
    out, _ = run(trace=False, **inputs)
    return out
